# revision 13
# baseline (speedup 1.0000x reference)
"""Trainium2 Bass kernel for nn_MultiHeadAttention_69466801045770.

Full-input contract: kernel(**inputs) takes the complete tensors and returns
the complete [B, T, D1] output. Internally:

  - 8 NeuronCores, core c -> (batch b = c//2, head-group g = c%2).
    Megatron-style tensor parallelism inside a batch: wq/wk/wv column-split,
    wo row-split; the two partial outputs per batch are summed on the host
    at gather time (the "AllReduce" of row-parallel linear).
  - Head group g owns global d_model columns [256g:256g+256] U
    [512+256g:512+256g+256] (heads {4g..4g+3, 8+4g..8+4g+3}), chosen so the
    reference's rotate_half RoPE pairs (i, i+512) stay inside one core.
  - Per core the device kernel computes, in bf16 matmuls / fp32 PSUM:
      qpT/kpT = (wq/wk)^T-projected activations in transposed [dcol, T]
      layout (+ bias + RoPE on the vector engine), vp in natural [s, dv]
      layout, then S^T = K Q^T per head (row-packed 2 heads per PE pass,
      K=64), exp on the scalar engine (scale 1/8 folded into ACTIVATE,
      PSUM->SBUF bf16), O^T = V^T @ expS^T (col-packed 2 heads per pass),
      softmax denominators as ones-vector matmuls (4 heads packed per pass),
      normalization via a tiny K=4 broadcast matmul + DVE multiply, and the
      final wo projection.
  - Softmax max-subtraction is omitted: scores for this operator are
    |s| <= ~3 (weights scaled by 0.02), exp() is exact-safe there and the
    reference's max-subtraction is mathematically a no-op.
  - The multiplicative all-ones mask is a no-op and skipped on device; a
    numpy fallback handles the general case. Zero-effect biases (bv, bo)
    are folded in exactly on the host: P@  (vp+bv) = P@vp + bv since the
    softmax rows sum to 1, so out += (bv@wo + bo).
"""

import numpy as np
import ml_dtypes

import bass_rust
import concourse.bass as bass
import concourse.mybir as mybir
import concourse.tile as tile
from concourse.vector_clock import ScopedClock
from concourse.bass_utils import run_bass_kernel_spmd

F32 = mybir.dt.float32
BF16 = mybir.dt.bfloat16
NPBF16 = ml_dtypes.bfloat16
ALU = mybir.AluOpType
ACTF = mybir.ActivationFunctionType

B, T, D1, D2, H = 4, 2048, 1024, 768, 16
DT = D1 // H          # 64 per-head dim
DL = D1 // 2          # 512 local d_model columns per core
N_CORES = 8
TC = 512              # t-chunk (PE moving free dim / PSUM bank)
NCHUNK = T // TC      # 4
NSB = T // 128        # 16 s-blocks
KQ = D1 // 128        # 8 din blocks for q
KK = D2 // 128        # 6 din blocks for k/v

TRACE = False          # set by test.py to collect an NTFF profile
LAST_RESULTS = None    # BassKernelResults of the last run (for test.py)

_NC = None             # cached compiled Bass module


def _split_tail_drain(self, tick_clock, wait_clock):
    """TileContext tail drain, split to one semaphore wait per Drain.

    The walrus build in this container rejects >1 sync-wait command on a
    CTRL (Drain) instruction; the stock tail drain carries one wait per
    outstanding DMA queue.
    """
    drain_inst = self.nc.sync.drain()
    wait_clock.add_sem_waits(
        drain_inst.ins, ScopedClock({None: tick_clock.global_clock})
    )
    si = drain_inst.ins.sync_info
    if si is not None and si.on_wait is not None and len(si.on_wait) > 1:
        waits = list(si.on_wait)
        si.on_wait = waits[:1]
        for w in waits[1:]:
            extra = self.nc.sync.drain()
            esi = extra.ins.sync_info
            if esi is None:
                extra.ins.sync_info = bass_rust.SyncInfo(on_wait=[w], on_update=[])
            else:
                esi.on_wait = [w]
    self.nc.all_engine_barrier()
    popped = self.nc._tile_sem_poison_stack.pop()
    assert popped is self._sem_poison
    self.nc.clear_and_free_semaphores(list(self.sems.allocated().values()))
    self.nc.all_engine_barrier()


tile.TileContext._drain_and_barrier = _split_tail_drain

_orig_commit = tile.TileContext._commit_instruction


def _commit_split_waits(self, inst, lazy_reg_writes=True):
    """Keep at most one sync wait per instruction (same walrus limit as the
    tail drain): move extra waits onto dedicated same-engine NOPs emitted
    just before the instruction, which block the engine queue equivalently.
    """
    si = inst.sync_info
    if (
        si is not None
        and si.on_wait is not None
        and len(si.on_wait) > 1
        and inst.engine != mybir.EngineType.Unassigned
    ):
        waits = list(si.on_wait)
        si.on_wait = waits[:1]
        for i, w in enumerate(waits[1:]):
            nop = mybir.InstNoOp(name=f"{inst.name}-ws{i}", ins=[], outs=[])
            nop.engine = inst.engine
            nop.bass_nofuse = True
            nop.sync_info = bass_rust.SyncInfo(on_wait=[w], on_update=[])
            self._add_instruction(nop)
    return _orig_commit(self, inst, lazy_reg_writes)


tile.TileContext._commit_instruction = _commit_split_waits


def _build_nc(rep=1):
    """Build the per-core program; rep>1 repeats the whole body (timing aid)."""
    nc = bass.Bass()

    qT = nc.declare_dram_parameter("qT", [D1, T], BF16, isOutput=False)
    kT = nc.declare_dram_parameter("kT", [D2, T], BF16, isOutput=False)
    vT = nc.declare_dram_parameter("vT", [D2, T], BF16, isOutput=False)
    wq = nc.declare_dram_parameter("wq", [D1, DL], BF16, isOutput=False)
    wk = nc.declare_dram_parameter("wk", [D2, DL], BF16, isOutput=False)
    wv = nc.declare_dram_parameter("wv", [D2, DL], BF16, isOutput=False)
    wo = nc.declare_dram_parameter("wo", [DL, D1], BF16, isOutput=False)
    cosT = nc.declare_dram_parameter("cosT", [256, T], F32, isOutput=False)
    sinT = nc.declare_dram_parameter("sinT", [256, T], F32, isOutput=False)
    bqT = nc.declare_dram_parameter("bqT", [128, 4], F32, isOutput=False)
    bkT = nc.declare_dram_parameter("bkT", [128, 4], F32, isOutput=False)
    sel = nc.declare_dram_parameter("sel", [1, 256], F32, isOutput=False)
    ones = nc.declare_dram_parameter("ones", [128, 1], BF16, isOutput=False)
    out = nc.declare_dram_parameter("out", [T, D1], F32, isOutput=True)

    with tile.TileContext(nc) as tc:
      for _rep in range(rep):
        with (
            # -------- SBUF pools --------
            tc.tile_pool(name="consts", bufs=1) as consts,      # weights/rope/bias
            tc.tile_pool(name="qstream", bufs=9) as qstream,    # qT din tiles
            tc.tile_pool(name="kstream", bufs=7) as kstream,
            tc.tile_pool(name="vstream", bufs=7) as vstream,
            tc.tile_pool(name="persist", bufs=1) as persist,    # roped qpT/kpT, vp, O_n
            tc.tile_pool(name="praw", bufs=5) as praw,          # fp32 proj staging
            tc.tile_pool(name="rtmp", bufs=4) as rtmp,          # rope temporaries
            tc.tile_pool(name="expp", bufs=2) as expp,          # exp(S^T) quad tiles
            tc.tile_pool(name="smalls", bufs=4) as smalls,      # recip tiles
            tc.tile_pool(name="ostage", bufs=2) as ostage,      # output staging
            # -------- PSUM pools (8 banks total) --------
            tc.tile_pool(name="scorep", bufs=1, space="PSUM") as scorep,  # 4 banks
            tc.tile_pool(name="avp", bufs=2, space="PSUM") as avp,        # 2 banks
            tc.tile_pool(name="mmp", bufs=2, space="PSUM") as mmp,        # 2 banks
        ):
            # ---- load constants ----
            wq_t = [consts.tile([128, DL], BF16, name=f"wq{d}") for d in range(KQ)]
            for d in range(KQ):
                nc.sync.dma_start(wq_t[d][:], wq[128 * d:128 * (d + 1), :])
            wk_t = [consts.tile([128, DL], BF16, name=f"wk{d}") for d in range(KK)]
            wv_t = [consts.tile([128, DL], BF16, name=f"wv{d}") for d in range(KK)]
            for d in range(KK):
                nc.sync.dma_start(wk_t[d][:], wk[128 * d:128 * (d + 1), :])
                nc.sync.dma_start(wv_t[d][:], wv[128 * d:128 * (d + 1), :])
            wo_t = [consts.tile([128, D1], BF16, name=f"wo{j}") for j in range(4)]
            for j in range(4):
                nc.sync.dma_start(wo_t[j][:], wo[128 * j:128 * (j + 1), :])
            cos_t = [consts.tile([128, T], F32, name=f"cos{j}") for j in range(2)]
            sin_t = [consts.tile([128, T], F32, name=f"sin{j}") for j in range(2)]
            for j in range(2):
                nc.sync.dma_start(cos_t[j][:], cosT[128 * j:128 * (j + 1), :])
                nc.sync.dma_start(sin_t[j][:], sinT[128 * j:128 * (j + 1), :])
            bq_t = consts.tile([128, 4], F32)
            bk_t = consts.tile([128, 4], F32)
            nc.sync.dma_start(bq_t[:], bqT[:])
            nc.sync.dma_start(bk_t[:], bkT[:])
            sel_t = consts.tile([1, 256], F32)
            nc.sync.dma_start(sel_t[:], sel[:])
            ones_t = consts.tile([128, 1], BF16)
            nc.sync.dma_start(ones_t[:], ones[:])

            # ---- persistent products ----
            qpT = [persist.tile([128, T], BF16, name=f"qpT{j}") for j in range(4)]
            kpT = [persist.tile([128, T], BF16, name=f"kpT{j}") for j in range(4)]
            vp = [persist.tile([128, DL], BF16, name=f"vp{s}") for s in range(NSB)]
            On = [persist.tile([128, T], BF16, name=f"On{j}") for j in range(4)]

            # ================= projections + RoPE =================
            def project_pair(raw, dst, j, cs, bias_t, cos_j, sin_j):
                """RoPE pair (j, j+2) of fp32 SBUF tiles -> bf16 dst chunks.

                out0 = (x0+b0)*cos - (x1+b1)*sin
                out1 = (x1+b1)*cos + (x0+b0)*sin
                """
                x0, x1 = raw[j], raw[j + 2]
                b0, b1 = bias_t[:, j:j + 1], bias_t[:, j + 2:j + 3]
                sl = (slice(None), slice(TC * cs, TC * (cs + 1)))
                t1 = rtmp.tile([128, TC], F32, tag="rt")
                nc.vector.scalar_tensor_tensor(
                    t1[:], x0[:], b0, cos_j, op0=ALU.add, op1=ALU.mult)
                t2 = rtmp.tile([128, TC], F32, tag="rt")
                nc.vector.scalar_tensor_tensor(
                    t2[:], x1[:], b1, sin_j, op0=ALU.add, op1=ALU.mult)
                nc.vector.tensor_sub(dst[j][sl], t1[:], t2[:])
                t3 = rtmp.tile([128, TC], F32, tag="rt")
                nc.vector.scalar_tensor_tensor(
                    t3[:], x1[:], b1, cos_j, op0=ALU.add, op1=ALU.mult)
                t4 = rtmp.tile([128, TC], F32, tag="rt")
                nc.vector.scalar_tensor_tensor(
                    t4[:], x0[:], b0, sin_j, op0=ALU.add, op1=ALU.mult)
                nc.vector.tensor_add(dst[j + 2][sl], t3[:], t4[:])

            for cs in range(NCHUNK):
                csl = slice(TC * cs, TC * (cs + 1))
                # stream this chunk of kT/vT/qT
                k_in = [kstream.tile([128, TC], BF16, tag="k", name=f"kin{cs}_{d}") for d in range(KK)]
                v_in = [vstream.tile([128, TC], BF16, tag="v", name=f"vin{cs}_{d}") for d in range(KK)]
                for d in range(KK):
                    nc.sync.dma_start(k_in[d][:], kT[128 * d:128 * (d + 1), csl])
                    nc.sync.dma_start(v_in[d][:], vT[128 * d:128 * (d + 1), csl])
                q_in = [qstream.tile([128, TC], BF16, tag="q", name=f"qin{cs}_{d}") for d in range(KQ)]
                for d in range(KQ):
                    nc.sync.dma_start(q_in[d][:], qT[128 * d:128 * (d + 1), csl])

                # kpT: accumulate in one PSUM slot, stage to fp32 SBUF, rope
                k_raw, q_raw = {}, {}
                for j in range(4):
                    ps = mmp.tile([128, TC], F32, tag="mm")
                    for d in range(KK):
                        nc.tensor.matmul(
                            ps[:], wk_t[d][:, 128 * j:128 * (j + 1)], k_in[d][:],
                            start=(d == 0), stop=(d == KK - 1))
                    r = praw.tile([128, TC], F32, tag="praw")
                    nc.vector.tensor_copy(r[:], ps[:])
                    k_raw[j] = r
                for j in range(2):
                    project_pair(k_raw, kpT, j, cs, bk_t,
                                 cos_t[j][:, csl], sin_t[j][:, csl])

                # vp: natural [s, dv] layout
                for ss in range(4):
                    s_idx = 4 * cs + ss
                    ps = mmp.tile([128, TC], F32, tag="mm")
                    for d in range(KK):
                        nc.tensor.matmul(
                            ps[:], v_in[d][:, 128 * ss:128 * (ss + 1)], wv_t[d][:],
                            start=(d == 0), stop=(d == KK - 1))
                    nc.vector.tensor_copy(vp[s_idx][:], ps[:])

                # qpT
                for j in range(4):
                    ps = mmp.tile([128, TC], F32, tag="mm")
                    for d in range(KQ):
                        nc.tensor.matmul(
                            ps[:], wq_t[d][:, 128 * j:128 * (j + 1)], q_in[d][:],
                            start=(d == 0), stop=(d == KQ - 1))
                    r = praw.tile([128, TC], F32, tag="praw")
                    nc.vector.tensor_copy(r[:], ps[:])
                    q_raw[j] = r
                for j in range(2):
                    project_pair(q_raw, qpT, j, cs, bq_t,
                                 cos_t[j][:, csl], sin_t[j][:, csl])

            # ================= attention =================
            # per (quad of 4 heads, t-chunk): s-loop of
            #   scores S^T -> exp -> O^T accumulation + column sums
            for qd in range(2):
                jA, jB = 2 * qd, 2 * qd + 1      # head-pair tiles of this quad
                for cs in range(NCHUNK):
                    csl = slice(TC * cs, TC * (cs + 1))
                    av = [avp.tile([128, TC], F32, tag="av", name=f"av{qd}_{cs}_{p}") for p in range(2)]
                    colsum = mmp.tile([128, TC], F32, tag="mm")
                    for sb in range(NSB):
                        ssl = slice(128 * sb, 128 * (sb + 1))
                        sc = scorep.tile([128, 4 * TC], F32, tag="sc")
                        # S^T tiles: 4 heads row-packed in pairs (K=64)
                        for i, (jj, rows) in enumerate(
                            ((jA, slice(0, 64)), (jA, slice(64, 128)),
                             (jB, slice(0, 64)), (jB, slice(64, 128)))):
                            nc.tensor.matmul(
                                sc[:, TC * i:TC * (i + 1)],
                                kpT[jj][rows, ssl], qpT[jj][rows, csl],
                                start=True, stop=True)
                        # exp of all 4 heads in one ACTIVATE, scale = 1/sqrt(DT)
                        ex = expp.tile([128, 4 * TC], BF16, tag="exp")
                        nc.scalar.activation(ex[:], sc[:], ACTF.Exp, scale=0.125)
                        # O^T accumulation: pairs col-packed; V stationary
                        for i in range(4):
                            p, hi = divmod(i, 2)
                            nc.tensor.matmul(
                                av[p][64 * hi:64 * (hi + 1), :],
                                vp[sb][:, 64 * (4 * qd + i):64 * (4 * qd + i + 1)],
                                ex[:, TC * i:TC * (i + 1)],
                                start=(sb == 0), stop=(sb == NSB - 1))
                        # column sums, 4 heads packed at partitions 0/32/64/96
                        for i in range(4):
                            nc.tensor.matmul(
                                colsum[32 * i:32 * i + 1, :],
                                ones_t[:], ex[:, TC * i:TC * (i + 1)],
                                start=(sb == 0), stop=(sb == NSB - 1),
                                tile_position=(0, 32 * i))
                    # normalize: per-head reciprocal of the packed colsums
                    # (partition-0-based [1, TC] tiles: DVE writes must start
                    # at partition 0), then broadcast across the 64 rows of
                    # each head via K=1 accumulated matmuls, then multiply.
                    recip = [smalls.tile([1, TC], F32, tag="recip",
                                         name=f"rc{qd}_{cs}_{i}") for i in range(4)]
                    for i in range(4):
                        nc.vector.reciprocal(
                            recip[i][:], colsum[32 * i:32 * i + 1, :])
                    for p, jj in enumerate((jA, jB)):
                        bc = mmp.tile([128, TC], F32, tag="mm")
                        nc.tensor.matmul(bc[:], sel_t[:, 0:128],
                                         recip[2 * p][:], start=True, stop=False)
                        nc.tensor.matmul(bc[:], sel_t[:, 128:256],
                                         recip[2 * p + 1][:], start=False, stop=True)
                        # DVE reads at most one PSUM operand: bounce bc to SBUF
                        bc_s = rtmp.tile([128, TC], F32, tag="rt",
                                         name=f"bcs{qd}_{cs}_{p}")
                        nc.vector.tensor_copy(bc_s[:], bc[:])
                        nc.vector.tensor_mul(On[jj][:, csl], av[p][:], bc_s[:])

            # ================= output projection =================
            for tb in range(NSB):
                tsl = slice(128 * tb, 128 * (tb + 1))
                st = ostage.tile([128, D1], F32, tag="ost")
                for half in range(2):
                    ps = mmp.tile([128, TC], F32, tag="mm")
                    for j in range(4):
                        nc.tensor.matmul(
                            ps[:], On[j][:, tsl],
                            wo_t[j][:, TC * half:TC * (half + 1)],
                            start=(j == 0), stop=(j == 3))
                    nc.vector.tensor_copy(st[:, TC * half:TC * (half + 1)], ps[:])
                nc.sync.dma_start(out[tsl, :], st[:])

    return nc


def _rope_cache_cols(g):
    """cos/sin for this core's first-half columns, [256, T] fp32 transposed."""
    inv_freq = 1.0 / (10000.0 ** (np.arange(0, D1, 2, dtype=np.float64) / D1))
    ang = np.arange(T, dtype=np.float64)[:, None] * inv_freq[None, :]  # [T, 512]
    sl = slice(256 * g, 256 * (g + 1))
    return (np.cos(ang[:, sl]).T.astype(np.float32),
            np.sin(ang[:, sl]).T.astype(np.float32))


def _numpy_fallback(q, k, v, mask, wq, bq, wk, bk, wv, bv, wo, bo):
    qp = q @ wq + bq
    kp = k @ wk + bk
    vp = v @ wv + bv
    inv_freq = 1.0 / (10000.0 ** (np.arange(0, D1, 2, dtype=np.float32) / D1))
    ang = np.arange(T, dtype=np.float32)[:, None] * inv_freq[None, :]
    emb = np.concatenate((ang, ang), axis=-1)
    cos, sin = np.cos(emb), np.sin(emb)

    def rot(x):
        x1, x2 = np.split(x, 2, axis=-1)
        return np.concatenate((-x2, x1), axis=-1)

    qp = qp * cos + rot(qp) * sin
    kp = kp * cos + rot(kp) * sin

    def heads(x):
        return x.reshape(B, T, H, DT).transpose(0, 2, 1, 3)

    qh, kh, vh = heads(qp), heads(kp), heads(vp)
    out = np.empty((B, H, T, DT), np.float32)
    for b in range(B):
        for h in range(H):
            s = (qh[b, h] @ kh[b, h].T) / np.sqrt(np.float32(DT))
            s = s * mask[b]
            e = np.exp(s - s.max(-1, keepdims=True))
            out[b, h] = (e / e.sum(-1, keepdims=True)) @ vh[b, h]
    out = out.transpose(0, 2, 1, 3).reshape(B, T, D1)
    return out @ wo + bo


def kernel(**inputs):
    global _NC, LAST_RESULTS
    q = np.asarray(inputs["q"], np.float32)
    k = np.asarray(inputs["k"], np.float32)
    v = np.asarray(inputs["v"], np.float32)
    mask = np.asarray(inputs["mask"], np.float32)
    wq = np.asarray(inputs["wq"], np.float32)
    bq = np.asarray(inputs["bq"], np.float32)
    wk = np.asarray(inputs["wk"], np.float32)
    bk = np.asarray(inputs["bk"], np.float32)
    wv = np.asarray(inputs["wv"], np.float32)
    bv = np.asarray(inputs["bv"], np.float32)
    wo = np.asarray(inputs["wo"], np.float32)
    bo = np.asarray(inputs["bo"], np.float32)

    if not np.all(mask == 1.0):
        return _numpy_fallback(q, k, v, mask, wq, bq, wk, bk, wv, bv, wo, bo)

    if _NC is None:
        _NC = _build_nc()

    in_maps = _prepare_in_maps(q, k, v, wq, bq, wk, bk, wv, wo)

    res = run_bass_kernel_spmd(_NC, in_maps, list(range(N_CORES)), trace=TRACE)
    LAST_RESULTS = res

    extra = bv @ wo + bo  # exact fold of the zero-effect biases (see docstring)
    out = np.empty((B, T, D1), np.float32)
    for b in range(B):
        out[b] = res.results[2 * b]["out"] + res.results[2 * b + 1]["out"] + extra
    return out


def _prepare_in_maps(q, k, v, wq, bq, wk, bk, wv, wo):
    # sel[0, 0:128] selects rows 0:64, sel[0, 128:256] selects rows 64:128:
    # lhsT columns of the K=1 normalization broadcast matmuls
    sel = np.zeros((1, 256), np.float32)
    sel[0, 0:64] = 1.0
    sel[0, 192:256] = 1.0

    in_maps = []
    for c in range(N_CORES):
        b, g = divmod(c, 2)
        cols = np.r_[256 * g:256 * (g + 1), 512 + 256 * g:512 + 256 * (g + 1)]
        cosT, sinT = _rope_cache_cols(g)
        in_maps.append({
            "qT": np.ascontiguousarray(q[b].T).astype(NPBF16),
            "kT": np.ascontiguousarray(k[b].T).astype(NPBF16),
            "vT": np.ascontiguousarray(v[b].T).astype(NPBF16),
            "wq": np.ascontiguousarray(wq[:, cols]).astype(NPBF16),
            "wk": np.ascontiguousarray(wk[:, cols]).astype(NPBF16),
            "wv": np.ascontiguousarray(wv[:, cols]).astype(NPBF16),
            "wo": np.ascontiguousarray(wo[cols, :]).astype(NPBF16),
            "cosT": cosT,
            "sinT": sinT,
            "bqT": np.ascontiguousarray(bq[cols].reshape(4, 128).T),
            "bkT": np.ascontiguousarray(bk[cols].reshape(4, 128).T),
            "sel": sel,
            "ones": np.ones((128, 1), NPBF16),
        })
    return in_maps


# revision 46
# speedup vs baseline: 2.6953x; 2.6953x over previous
"""Trainium2 Bass kernel for nn_MultiHeadAttention_69466801045770.

Full-input contract: kernel(**inputs) takes the complete tensors and returns
the complete [B, T, D1] output. Internally:

  - 8 NeuronCores, core c -> (batch b = c//2, head-group g = c%2).
    Megatron-style tensor parallelism inside a batch: wq/wk/wv column-split,
    wo row-split; the two partial outputs per batch are summed on the host
    at gather time (the "AllReduce" of row-parallel linear).
  - Head group g owns global d_model columns [256g:256g+256] U
    [512+256g:512+256g+256] (heads {4g..4g+3, 8+4g..8+4g+3}), chosen so the
    reference's rotate_half RoPE pairs (i, i+512) stay inside one core.
  - Per core the device kernel computes, in bf16 matmuls / fp32 PSUM:
      qpT/kpT = (wq/wk)^T-projected activations in transposed [dcol, T]
      layout (+ bias + RoPE on the vector engine); vp in natural [s, dv]
      layout AUGMENTED with a ones column per head (65 cols/head) so that
      the attention-value matmul's 65th output row accumulates the softmax
      denominator for free; then per (head-pair, t-chunk, s-block):
      S^T = K Q^T (2 heads row-packed per PE pass, K=64, into a 2-bank
      PSUM tile from a 2-slot pool so PE and ACT ping-pong), exp on the
      scalar engine (scale 1/sqrt(64) folded into ACTIVATE, PSUM->SBUF
      bf16), O_aug^T accumulation with V_aug stationary (M=65, N=512);
      normalization = reciprocal of the denominator row + K=1 ones-matmul
      broadcast across the head's 64 rows + DVE multiply; finally the wo
      projection with O_n^T as the stationary operand.
  - Softmax max-subtraction is omitted: scores for this operator are
    |s| <= ~3 (weights scaled by 0.02), exp() is exact-safe there and the
    reference's max-subtraction is mathematically a no-op.
  - The multiplicative all-ones mask is a no-op and skipped on device; a
    numpy fallback handles the general case. Zero-effect biases (bv, bo)
    are folded in exactly on the host: P@  (vp+bv) = P@vp + bv since the
    softmax rows sum to 1, so out += (bv@wo + bo).
"""

import numpy as np
import ml_dtypes

import bass_rust
import concourse.bass as bass
import concourse.mybir as mybir
import concourse.tile as tile
from concourse.vector_clock import ScopedClock
from concourse.bass_utils import run_bass_kernel_spmd

F32 = mybir.dt.float32
F32R = mybir.dt.float32r
BF16 = mybir.dt.bfloat16
NPBF16 = ml_dtypes.bfloat16
ALU = mybir.AluOpType
ACTF = mybir.ActivationFunctionType

B, T, D1, D2, H = 4, 2048, 1024, 768, 16
DT = D1 // H          # 64 per-head dim
DL = D1 // 2          # 512 local d_model columns per core
N_CORES = 8
TC = 512              # t-chunk (PE moving free dim / PSUM bank)
NCHUNK = T // TC      # 4
NSB = T // 128        # 16 s-blocks
KQ = D1 // 128        # 8 din blocks for q
KK = D2 // 128        # 6 din blocks for k/v

TRACE = False          # set by test.py to collect an NTFF profile
LAST_RESULTS = None    # BassKernelResults of the last run (for test.py)

_NC = None             # cached compiled Bass module


def _split_tail_drain(self, tick_clock, wait_clock):
    """TileContext tail drain, split to one semaphore wait per Drain.

    The walrus build in this container rejects >1 sync-wait command on a
    CTRL (Drain) instruction; the stock tail drain carries one wait per
    outstanding DMA queue.
    """
    drain_inst = self.nc.sync.drain()
    wait_clock.add_sem_waits(
        drain_inst.ins, ScopedClock({None: tick_clock.global_clock})
    )
    si = drain_inst.ins.sync_info
    if si is not None and si.on_wait is not None and len(si.on_wait) > 1:
        waits = list(si.on_wait)
        si.on_wait = waits[:1]
        for w in waits[1:]:
            extra = self.nc.sync.drain()
            esi = extra.ins.sync_info
            if esi is None:
                extra.ins.sync_info = bass_rust.SyncInfo(on_wait=[w], on_update=[])
            else:
                esi.on_wait = [w]
    self.nc.all_engine_barrier()
    popped = self.nc._tile_sem_poison_stack.pop()
    assert popped is self._sem_poison
    self.nc.clear_and_free_semaphores(list(self.sems.allocated().values()))
    self.nc.all_engine_barrier()


tile.TileContext._drain_and_barrier = _split_tail_drain

# idempotent under module reload: keep the true original on the class
if not hasattr(tile.TileContext, "_ant_orig_commit"):
    tile.TileContext._ant_orig_commit = tile.TileContext._commit_instruction
_orig_commit = tile.TileContext._ant_orig_commit


def _commit_split_waits(self, inst, lazy_reg_writes=True):
    """Keep at most one sync wait per instruction (same walrus limit as the
    tail drain): move extra waits onto dedicated same-engine NOPs emitted
    just before the instruction, which block the engine queue equivalently.
    """
    si = inst.sync_info
    if (
        si is not None
        and si.on_wait is not None
        and len(si.on_wait) > 1
        and inst.engine != mybir.EngineType.Unassigned
    ):
        waits = list(si.on_wait)
        si.on_wait = waits[:1]
        for i, w in enumerate(waits[1:]):
            nop = mybir.InstNoOp(name=f"{inst.name}-ws{i}", ins=[], outs=[])
            nop.engine = inst.engine
            nop.bass_nofuse = True
            nop.sync_info = bass_rust.SyncInfo(on_wait=[w], on_update=[])
            self._add_instruction(nop)
    return _orig_commit(self, inst, lazy_reg_writes)


tile.TileContext._commit_instruction = _commit_split_waits


def _build_nc(rep=1, phase="full"):
    """Build the per-core program.

    rep>1 repeats the whole body (timing aid). phase in
    {"proj", "scores", "full"} truncates the pipeline (phase attribution).
    """
    nc = bass.Bass()

    qT = nc.declare_dram_parameter("qT", [D1, T], BF16, isOutput=False)
    kT = nc.declare_dram_parameter("kT", [D2, T], BF16, isOutput=False)
    vT = nc.declare_dram_parameter("vT", [D2, T], BF16, isOutput=False)
    wq = nc.declare_dram_parameter("wq", [D1, DL], BF16, isOutput=False)
    wk = nc.declare_dram_parameter("wk", [D2, DL], BF16, isOutput=False)
    wv = nc.declare_dram_parameter("wv", [D2, DL], BF16, isOutput=False)
    wo = nc.declare_dram_parameter("wo", [DL, D1], BF16, isOutput=False)
    cosT = nc.declare_dram_parameter("cosT", [256, T], F32, isOutput=False)
    sinT = nc.declare_dram_parameter("sinT", [256, T], F32, isOutput=False)
    bqT = nc.declare_dram_parameter("bqT", [128, 4], F32, isOutput=False)
    bkT = nc.declare_dram_parameter("bkT", [128, 4], F32, isOutput=False)
    sel = nc.declare_dram_parameter("sel", [1, 256], F32R, isOutput=False)
    ones = nc.declare_dram_parameter("ones", [128, 1], BF16, isOutput=False)
    out = nc.declare_dram_parameter("out", [T, D1], F32, isOutput=True)

    with tile.TileContext(nc) as tc:
      for _rep in range(rep):
        with (
            # -------- SBUF pools --------
            tc.tile_pool(name="consts", bufs=1) as consts,      # weights/rope/bias
            tc.tile_pool(name="qstream", bufs=2) as qstream,    # qT din tiles
            tc.tile_pool(name="kstream", bufs=2) as kstream,
            tc.tile_pool(name="vstream", bufs=2) as vstream,
            tc.tile_pool(name="persist", bufs=1) as persist,    # roped qpT/kpT, vp, O_n
            tc.tile_pool(name="praw", bufs=5) as praw,          # fp32 proj staging
            tc.tile_pool(name="rtmp", bufs=4) as rtmp,          # rope temporaries
            tc.tile_pool(name="expp", bufs=2) as expp,          # exp(S^T) quad tiles
            tc.tile_pool(name="smalls", bufs=4) as smalls,      # recip tiles
            tc.tile_pool(name="ostage", bufs=2) as ostage,      # output staging
            # -------- PSUM pools (8 banks total) --------
            tc.tile_pool(name="scorep", bufs=2, space="PSUM") as scorep,  # 4 banks
            tc.tile_pool(name="avp", bufs=2, space="PSUM") as avp,        # 2 banks
            tc.tile_pool(name="mmp", bufs=2, space="PSUM") as mmp,        # 2 banks
        ):
            # ---- load constants ----
            # one wide tile + one strided DMA per tensor (DMA queue-head
            # cost is per-descriptor, so merged loads beat per-block loads)
            wq_t = consts.tile([128, KQ * DL], BF16)
            wk_t = consts.tile([128, KK * DL], BF16)
            wv_t = consts.tile([128, KK * DL], BF16)
            nc.sync.dma_start(
                wk_t[:].rearrange("p (d c) -> p d c", c=DL),
                wk[:].rearrange("(d p) c -> p d c", p=128))
            nc.sync.dma_start(
                wv_t[:].rearrange("p (d c) -> p d c", c=DL),
                wv[:].rearrange("(d p) c -> p d c", p=128))
            wo_t = consts.tile([128, 4 * D1], BF16)
            cos_t = consts.tile([128, 2 * T], F32)
            sin_t = consts.tile([128, 2 * T], F32)
            bq_t = consts.tile([128, 4], F32)
            bk_t = consts.tile([128, 4], F32)
            sel_t = consts.tile([1, 256], F32R)
            ones_t = consts.tile([128, 1], BF16)

            def load_deferred_consts():
                # emitted after chunk-0's activation streams so the first
                # projection matmuls are not stuck behind these transfers
                nc.sync.dma_start(
                    cos_t[:].rearrange("p (j t) -> p j t", t=T),
                    cosT[:].rearrange("(j p) t -> p j t", p=128))
                nc.sync.dma_start(
                    sin_t[:].rearrange("p (j t) -> p j t", t=T),
                    sinT[:].rearrange("(j p) t -> p j t", p=128))
                nc.sync.dma_start(
                    wq_t[:].rearrange("p (d c) -> p d c", c=DL),
                    wq[:].rearrange("(d p) c -> p d c", p=128))
                nc.sync.dma_start(bq_t[:], bqT[:])
                nc.sync.dma_start(bk_t[:], bkT[:])
                nc.sync.dma_start(sel_t[:], sel[:])
                nc.sync.dma_start(ones_t[:], ones[:])
                nc.sync.dma_start(
                    wo_t[:].rearrange("p (j c) -> p j c", c=D1),
                    wo[:].rearrange("(j p) c -> p j c", p=128))

            # ---- persistent products ----
            qpT = [persist.tile([128, T], BF16, name=f"qpT{j}") for j in range(4)]
            kpT = [persist.tile([128, T], BF16, name=f"kpT{j}") for j in range(4)]
            # vp_aug: per head 64 V columns + a ones column (65 each) so the
            # AV matmul's 65th output row accumulates the softmax denominator
            vp = [persist.tile([128, DL + 8], BF16, name=f"vp{s}")
                  for s in range(NSB)]
            On = [persist.tile([128, T], BF16, name=f"On{j}") for j in range(4)]

            # ================= projections + RoPE =================
            def project_pair(raw, dst, j, cs, bias_t, cos_j, sin_j):
                """RoPE pair (j, j+2) of fp32 SBUF tiles -> bf16 dst chunks.

                out0 = (x0+b0)*cos - (x1+b1)*sin
                out1 = (x1+b1)*cos + (x0+b0)*sin
                """
                x0, x1 = raw[j], raw[j + 2]
                b0, b1 = bias_t[:, j:j + 1], bias_t[:, j + 2:j + 3]
                sl = (slice(None), slice(TC * cs, TC * (cs + 1)))
                t1 = rtmp.tile([128, TC], F32, tag="rt")
                nc.vector.scalar_tensor_tensor(
                    t1[:], x0[:], b0, cos_j, op0=ALU.add, op1=ALU.mult)
                t2 = rtmp.tile([128, TC], F32, tag="rt")
                nc.vector.scalar_tensor_tensor(
                    t2[:], x1[:], b1, sin_j, op0=ALU.add, op1=ALU.mult)
                nc.vector.tensor_sub(dst[j][sl], t1[:], t2[:])
                t3 = rtmp.tile([128, TC], F32, tag="rt")
                nc.vector.scalar_tensor_tensor(
                    t3[:], x1[:], b1, cos_j, op0=ALU.add, op1=ALU.mult)
                t4 = rtmp.tile([128, TC], F32, tag="rt")
                nc.vector.scalar_tensor_tensor(
                    t4[:], x0[:], b0, sin_j, op0=ALU.add, op1=ALU.mult)
                nc.vector.tensor_add(dst[j + 2][sl], t3[:], t4[:])

            # ================= attention =================
            # per (head-pair tile jj, t-chunk): s-loop of S^T (2 heads
            # row-packed) -> exp -> O^T via V_aug-stationary matmul whose
            # 65th row accumulates the softmax denominator.
            def attend(jj, cs):
                csl = slice(TC * cs, TC * (cs + 1))
                av = [avp.tile([65, TC], F32, tag="av",
                               name=f"av{jj}_{cs}_{h}") for h in range(2)]
                for sb in range(NSB):
                    ssl = slice(128 * sb, 128 * (sb + 1))
                    sc = scorep.tile([128, 2 * TC], F32, tag="sc",
                                     name=f"sc{jj}_{cs}_{sb}")
                    ex = expp.tile([128, 2 * TC], BF16, tag="exp",
                                   name=f"ex{jj}_{cs}_{sb}")
                    for hi in range(2):
                        rows = slice(64 * hi, 64 * (hi + 1))
                        nc.tensor.matmul(
                            sc[:, TC * hi:TC * (hi + 1)],
                            kpT[jj][rows, ssl], qpT[jj][rows, csl],
                            start=True, stop=True)
                    nc.scalar.activation(ex[:], sc[:], ACTF.Exp, scale=0.125)
                    if phase == "scores":
                        continue
                    for hi in range(2):
                        lh = 2 * jj + hi     # local head index
                        nc.tensor.matmul(
                            av[hi][:, :],
                            vp[sb][:, 65 * lh:65 * (lh + 1)],
                            ex[:, TC * hi:TC * (hi + 1)],
                            start=(sb == 0), stop=(sb == NSB - 1))
                if phase == "scores":
                    nc.vector.tensor_copy(On[jj][0:1, csl], ex[0:1, 0:TC])
                    return
                # normalize: reciprocal of the denominator row, broadcast
                # across the head's 64 rows via a K=1 matmul, multiply
                for hi in range(2):
                    recip = smalls.tile([1, TC], F32R, tag="recip",
                                        name=f"rc{jj}_{cs}_{hi}")
                    # fp32r is bit-identical storage; the dtype tag satisfies
                    # the verifier's fp32r-producer rule for the K=1 matmul
                    with nc.allow_low_precision(reason="fp32r bcast matmul"):
                        nc.vector.reciprocal(recip[:], av[hi][64:65, :])
                    av_s = rtmp.tile([64, TC], F32, tag="rt",
                                     name=f"avs{jj}_{cs}_{hi}")
                    nc.vector.tensor_copy(av_s[:], av[hi][0:64, :])
                    bc = mmp.tile([64, TC], F32, tag="mm",
                                  name=f"bc{jj}_{cs}_{hi}")
                    nc.tensor.matmul(bc[:], sel_t[:, 0:64], recip[:],
                                     start=True, stop=True)
                    nc.vector.tensor_mul(
                        On[jj][64 * hi:64 * (hi + 1), csl],
                        av_s[:], bc[:])

            for cs in range(NCHUNK):
                csl = slice(TC * cs, TC * (cs + 1))
                k_in = kstream.tile([128, KK * TC], BF16, tag="k",
                                    name=f"kin{cs}")
                v_in = vstream.tile([128, KK * TC], BF16, tag="v",
                                    name=f"vin{cs}")
                q_in = qstream.tile([128, KQ * TC], BF16, tag="q",
                                    name=f"qin{cs}")
                nc.sync.dma_start(
                    k_in[:].rearrange("p (d t) -> p d t", t=TC),
                    kT[:, csl].rearrange("(d p) t -> p d t", p=128))
                nc.sync.dma_start(
                    v_in[:].rearrange("p (d t) -> p d t", t=TC),
                    vT[:, csl].rearrange("(d p) t -> p d t", p=128))
                nc.sync.dma_start(
                    q_in[:].rearrange("p (d t) -> p d t", t=TC),
                    qT[:, csl].rearrange("(d p) t -> p d t", p=128))
                if cs == 0:
                    load_deferred_consts()

                # kpT: accumulate in one PSUM slot, stage to fp32 SBUF, rope
                k_raw, q_raw = {}, {}
                for j in range(4):
                    ps = mmp.tile([128, TC], F32, tag="mm")
                    for d in range(KK):
                        nc.tensor.matmul(
                            ps[:],
                            wk_t[:, DL * d + 128 * j:DL * d + 128 * (j + 1)],
                            k_in[:, TC * d:TC * (d + 1)],
                            start=(d == 0), stop=(d == KK - 1))
                    r = praw.tile([128, TC], F32, tag="praw")
                    nc.vector.tensor_copy(r[:], ps[:])
                    k_raw[j] = r
                for j in range(2):
                    project_pair(k_raw, kpT, j, cs, bk_t,
                                 cos_t[:, T * j + TC * cs:T * j + TC * (cs + 1)],
                                 sin_t[:, T * j + TC * cs:T * j + TC * (cs + 1)])

                # vp_aug: natural [s, dv] layout + ones columns
                for ss in range(4):
                    s_idx = 4 * cs + ss
                    ps = mmp.tile([128, TC], F32, tag="mm")
                    for d in range(KK):
                        nc.tensor.matmul(
                            ps[:],
                            v_in[:, TC * d + 128 * ss:TC * d + 128 * (ss + 1)],
                            wv_t[:, DL * d:DL * (d + 1)],
                            start=(d == 0), stop=(d == KK - 1))
                    nc.vector.tensor_copy(
                        vp[s_idx][:].rearrange("p (h e) -> p h e", e=65)[:, :, 0:64],
                        ps[:].rearrange("p (h e) -> p h e", e=64))
                    nc.gpsimd.memset(
                        vp[s_idx][:].rearrange("p (h e) -> p h e", e=65)[:, :, 64:65],
                        1.0)

                # qpT
                for j in range(4):
                    ps = mmp.tile([128, TC], F32, tag="mm")
                    for d in range(KQ):
                        nc.tensor.matmul(
                            ps[:],
                            wq_t[:, DL * d + 128 * j:DL * d + 128 * (j + 1)],
                            q_in[:, TC * d:TC * (d + 1)],
                            start=(d == 0), stop=(d == KQ - 1))
                    r = praw.tile([128, TC], F32, tag="praw")
                    nc.vector.tensor_copy(r[:], ps[:])
                    q_raw[j] = r
                for j in range(2):
                    project_pair(q_raw, qpT, j, cs, bq_t,
                                 cos_t[:, T * j + TC * cs:T * j + TC * (cs + 1)],
                                 sin_t[:, T * j + TC * cs:T * j + TC * (cs + 1)])

            if phase == "proj":
                # phase-attribution build: flush a few tiles so nothing
                # upstream is dead-code-eliminated, then stop.
                for j in range(4):
                    nc.gpsimd.dma_start(out[128 * j:128 * (j + 1), :],
                                        qpT[j][:, 0:D1])
                    nc.gpsimd.dma_start(out[128 * (j + 4):128 * (j + 5), :],
                                        kpT[j][:, 0:D1])
                for s in range(8):
                    nc.gpsimd.dma_start(
                        out[128 * (s + 8):128 * (s + 8) + 64, 0:DL],
                        vp[s][0:64, :])
                continue

            # chunk-major: after all 4 pairs finish a t-chunk, its four
            # 128-row output-projection blocks run overlapped with the
            # attention of later chunks
            for cs in range(NCHUNK):
                for jj in range(4):
                    attend(jj, cs)
                if phase == "scores":
                    continue
                for tb in range(4 * cs, 4 * (cs + 1)):
                    tsl = slice(128 * tb, 128 * (tb + 1))
                    st = ostage.tile([128, D1], F32, tag="ost",
                                     name=f"st{tb}")
                    for half in range(2):
                        ps = mmp.tile([128, TC], F32, tag="mm")
                        for j in range(4):
                            nc.tensor.matmul(
                                ps[:], On[j][:, tsl],
                                wo_t[:, D1 * j + TC * half:
                                     D1 * j + TC * (half + 1)],
                                start=(j == 0), stop=(j == 3))
                        nc.vector.tensor_copy(
                            st[:, TC * half:TC * (half + 1)], ps[:])
                    nc.sync.dma_start(out[tsl, :], st[:])

    return nc


def _rope_cache_cols(g):
    """cos/sin for this core's first-half columns, [256, T] fp32 transposed."""
    inv_freq = 1.0 / (10000.0 ** (np.arange(0, D1, 2, dtype=np.float64) / D1))
    ang = np.arange(T, dtype=np.float64)[:, None] * inv_freq[None, :]  # [T, 512]
    sl = slice(256 * g, 256 * (g + 1))
    return (np.cos(ang[:, sl]).T.astype(np.float32),
            np.sin(ang[:, sl]).T.astype(np.float32))


def _numpy_fallback(q, k, v, mask, wq, bq, wk, bk, wv, bv, wo, bo):
    qp = q @ wq + bq
    kp = k @ wk + bk
    vp = v @ wv + bv
    inv_freq = 1.0 / (10000.0 ** (np.arange(0, D1, 2, dtype=np.float32) / D1))
    ang = np.arange(T, dtype=np.float32)[:, None] * inv_freq[None, :]
    emb = np.concatenate((ang, ang), axis=-1)
    cos, sin = np.cos(emb), np.sin(emb)

    def rot(x):
        x1, x2 = np.split(x, 2, axis=-1)
        return np.concatenate((-x2, x1), axis=-1)

    qp = qp * cos + rot(qp) * sin
    kp = kp * cos + rot(kp) * sin

    def heads(x):
        return x.reshape(B, T, H, DT).transpose(0, 2, 1, 3)

    qh, kh, vh = heads(qp), heads(kp), heads(vp)
    out = np.empty((B, H, T, DT), np.float32)
    for b in range(B):
        for h in range(H):
            s = (qh[b, h] @ kh[b, h].T) / np.sqrt(np.float32(DT))
            s = s * mask[b]
            e = np.exp(s - s.max(-1, keepdims=True))
            out[b, h] = (e / e.sum(-1, keepdims=True)) @ vh[b, h]
    out = out.transpose(0, 2, 1, 3).reshape(B, T, D1)
    return out @ wo + bo


def kernel(**inputs):
    global _NC, LAST_RESULTS
    q = np.asarray(inputs["q"], np.float32)
    k = np.asarray(inputs["k"], np.float32)
    v = np.asarray(inputs["v"], np.float32)
    mask = np.asarray(inputs["mask"], np.float32)
    wq = np.asarray(inputs["wq"], np.float32)
    bq = np.asarray(inputs["bq"], np.float32)
    wk = np.asarray(inputs["wk"], np.float32)
    bk = np.asarray(inputs["bk"], np.float32)
    wv = np.asarray(inputs["wv"], np.float32)
    bv = np.asarray(inputs["bv"], np.float32)
    wo = np.asarray(inputs["wo"], np.float32)
    bo = np.asarray(inputs["bo"], np.float32)

    if not np.all(mask == 1.0):
        return _numpy_fallback(q, k, v, mask, wq, bq, wk, bk, wv, bv, wo, bo)

    if _NC is None:
        _NC = _build_nc()

    in_maps = _prepare_in_maps(q, k, v, wq, bq, wk, bk, wv, wo)

    # the axon terminal occasionally reports NRT_EXEC_UNIT_UNRECOVERABLE on
    # the first execution of a freshly loaded NEFF and recovers on retry
    last_exc = None
    for _attempt in range(3):
        try:
            res = run_bass_kernel_spmd(
                _NC, in_maps, list(range(N_CORES)), trace=TRACE)
            break
        except Exception as exc:  # noqa: BLE001 - retry transient device errors
            last_exc = exc
    else:
        raise last_exc
    LAST_RESULTS = res

    extra = bv @ wo + bo  # exact fold of the zero-effect biases (see docstring)
    out = np.empty((B, T, D1), np.float32)
    for b in range(B):
        out[b] = res.results[2 * b]["out"] + res.results[2 * b + 1]["out"] + extra
    return out


def _prepare_in_maps(q, k, v, wq, bq, wk, bk, wv, wo):
    # sel[0, 0:128] selects rows 0:64, sel[0, 128:256] selects rows 64:128:
    # lhsT columns of the K=1 normalization broadcast matmuls
    sel = np.zeros((1, 256), np.float32)
    sel[0, 0:64] = 1.0
    sel[0, 192:256] = 1.0

    in_maps = []
    for c in range(N_CORES):
        b, g = divmod(c, 2)
        cols = np.r_[256 * g:256 * (g + 1), 512 + 256 * g:512 + 256 * (g + 1)]
        cosT, sinT = _rope_cache_cols(g)
        in_maps.append({
            "qT": np.ascontiguousarray(q[b].T).astype(NPBF16),
            "kT": np.ascontiguousarray(k[b].T).astype(NPBF16),
            "vT": np.ascontiguousarray(v[b].T).astype(NPBF16),
            "wq": np.ascontiguousarray(wq[:, cols]).astype(NPBF16),
            "wk": np.ascontiguousarray(wk[:, cols]).astype(NPBF16),
            "wv": np.ascontiguousarray(wv[:, cols]).astype(NPBF16),
            "wo": np.ascontiguousarray(wo[cols, :]).astype(NPBF16),
            "cosT": cosT,
            "sinT": sinT,
            "bqT": np.ascontiguousarray(bq[cols].reshape(4, 128).T),
            "bkT": np.ascontiguousarray(bk[cols].reshape(4, 128).T),
            "sel": sel,
            "ones": np.ones((128, 1), NPBF16),
        })
    return in_maps
